# revision 29
# baseline (speedup 1.0000x reference)
"""Trainium2 Bass kernel for the caption-image sparse attention similarity module.

Strategy (hardcoded for imgs (64,36,512), caps (64,40,512), 8 cores):
  - Shard caption axis across 8 cores (8 captions = 320 query words each);
    imgs + weights replicated. No collectives; host gathers per-core outputs.
  - Algebraic reduction: with the module's zero biases the LayerNorm sigma
    cancels from s = (out.q)/||out||, so the huge (Bc,Bi,W,D) ctx/out tensors
    are never materialized. Everything reduces to:
      sims = q k^T, T = q vt^T (vt = v @ diag(g4) Wo^T), e = exp(sims),
      per-image Gram G3 = vt vt^T, Y = e @ [G3 | m | p | rmask],
      s = (tq - mu*qw) / sqrt(c3 + mu^2*Omega - 2*mu*pp)
    where tq = sum_r e*T, c3 = sum_r e*Y3, and mu/pp/S come out of Y.
  - exp needs no max-subtraction: |sims| <= ||q||*||k||/sqrt(D) ~ 23 << 88.
"""
import os
import numpy as np

Bi, R, D = 64, 36, 512
Bc, W = 64, 40
N_CORES = 8
CS = Bc // N_CORES            # captions per core
M = CS * W                    # 320 cap rows per core
NPAIR = Bi // 2               # 32 image pairs
NI = Bi * R                   # 2304 image rows
LN_EPS = 1e-5
ZT_CAP0 = NPAIR * 128         # caps start col in zT (4096)
ZT_F = ZT_CAP0 + 3 * 128      # 4480
MT = [(0, 128), (128, 128), (256, 64)]            # cap-row tiles
ICH = [(0, 12), (12, 12), (24, 12), (36, 12), (48, 12), (60, 4)]  # img chunks

_cache = {}
last_results = None


def _build_nc():
    from contextlib import ExitStack
    import concourse.bass as bass
    import concourse.tile as tile
    from concourse import bacc, mybir

    f32 = mybir.dt.float32
    bf16 = mybir.dt.bfloat16
    AX = mybir.AxisListType
    OP = mybir.AluOpType
    AF = mybir.ActivationFunctionType

    nc = bacc.Bacc("TRN2", target_bir_lowering=False, debug=False,
                   enable_asserts=False, num_devices=N_CORES)

    d_img = nc.dram_tensor("imgs_bf", (NI, D), bf16, kind="ExternalInput").ap()
    d_cap = nc.dram_tensor("caps_bf", (M, D), bf16, kind="ExternalInput").ap()
    d_wk = nc.dram_tensor("wk", (D, D), bf16, kind="ExternalInput").ap()
    d_wvt = nc.dram_tensor("wvt", (D, D), bf16, kind="ExternalInput").ap()
    d_wq = nc.dram_tensor("wq", (D, D), bf16, kind="ExternalInput").ap()
    d_mpr = nc.dram_tensor("mpr", (R, Bi, 4), bf16, kind="ExternalInput").ap()
    d_rm = nc.dram_tensor("rmask", (NI, 1), f32, kind="ExternalInput").ap()
    d_nqw = nc.dram_tensor("nqw", (M, 1), f32, kind="ExternalInput").ap()
    d_cm = nc.dram_tensor("cmv", (M, 2), f32, kind="ExternalInput").ap()
    d_om = nc.dram_tensor("omega", (1, 1), f32, kind="ExternalInput").ap()
    d_out = nc.dram_tensor("out_s", (M, Bi), f32, kind="ExternalOutput").ap()
    stage = int(os.environ.get("BK_STAGE", "9"))
    d_dbg = None
    if stage != 9:
        d_dbg = nc.dram_tensor("dbg", (128, 512), f32, kind="ExternalOutput").ap()

    with tile.TileContext(nc) as tc, ExitStack() as ctx:
        sing = ctx.enter_context(tc.tile_pool(name="sing", bufs=1))
        zpool = ctx.enter_context(tc.tile_pool(name="zpool", bufs=6))
        stp = ctx.enter_context(tc.tile_pool(name="stp", bufs=8))
        big = ctx.enter_context(tc.tile_pool(name="big", bufs=2))
        fin = ctx.enter_context(tc.tile_pool(name="fin", bufs=2))
        ppool = ctx.enter_context(tc.tile_pool(name="ppool", bufs=4, space="PSUM"))
        gpool = ctx.enter_context(tc.tile_pool(name="gpool", bufs=1, space="PSUM"))
        ypool = ctx.enter_context(tc.tile_pool(name="ypool", bufs=2, space="PSUM"))

        # ---- persistent SBUF ----
        w_sb = {}
        for nm, dram in (("wk", d_wk), ("wvt", d_wvt), ("wq", d_wq)):
            t = sing.tile([128, 4, D], bf16, tag=f"w_{nm}")
            for kin in range(4):
                nc.gpsimd.dma_start(out=t[:, kin, :], in_=dram[128 * kin:128 * (kin + 1), :])
            w_sb[nm] = t
        zT = sing.tile([128, 4, ZT_F], bf16, tag="zT")
        kT = sing.tile([128, 4, NI], bf16, tag="kT")
        vtT = sing.tile([128, 4, NI], bf16, tag="vtT")
        qT = sing.tile([128, 4, M], bf16, tag="qT")
        eT = sing.tile([128, NPAIR, 384], bf16, tag="eT")
        # H rows live at partition 0 (even images) or 64 (odd) to match eT parity
        H = sing.tile([128, Bi, 40], bf16, tag="H")
        nc.gpsimd.dma_start(out=H[0:R, 0:Bi:2, 36:40], in_=d_mpr[:, 0:Bi:2, :])
        nc.gpsimd.dma_start(out=H[64:64 + R, 1:Bi:2, 36:40], in_=d_mpr[:, 1:Bi:2, :])
        eps_sb = sing.tile([128, 1], f32, tag="eps")
        nc.vector.memset(eps_sb, LN_EPS)
        eps12 = sing.tile([128, 1], f32, tag="eps12")
        nc.vector.memset(eps12, 1e-12)
        om_sb = sing.tile([128, 1], f32, tag="om")
        nc.gpsimd.dma_start(
            out=om_sb,
            in_=bass.AP(tensor=d_om.tensor, offset=d_om.offset, ap=[[0, 128], [1, 1]]))
        qw_sb = sing.tile([128, 3, 1], f32, tag="qw")
        cm_sb = sing.tile([128, 3, 2], f32, tag="cm")
        for mt, (m0, mc) in enumerate(MT):
            nc.gpsimd.dma_start(out=qw_sb[0:mc, mt, :], in_=d_nqw[m0:m0 + mc, :])
            nc.gpsimd.dma_start(out=cm_sb[0:mc, mt, :], in_=d_cm[m0:m0 + mc, :])
        stat3 = sing.tile([128, 3, Bi, 3], f32, tag="stat3")
        c3u = sing.tile([128, 3, Bi], f32, tag="c3u")
        tqu = sing.tile([128, 3, Bi], f32, tag="tqu")

        # ---- LN + transpose (imgs as 72-row pairs, caps as 128-row tiles) ----
        def ln_tile(x, rows, masked_rm):
            bn6 = stp.tile([128, 6], mybir.dt.float32, tag="bn6")
            nc.vector.bn_stats(bn6[0:rows], x[0:rows, :])
            mv = stp.tile([128, 2], mybir.dt.float32, tag="mv")
            nc.vector.bn_aggr(mv[0:rows], bn6[0:rows])
            rs = stp.tile([128, 1], mybir.dt.float32, tag="rs")
            nc.scalar.activation(rs[0:rows], mv[0:rows, 1:2], AF.Sqrt,
                                 bias=eps_sb[0:rows], scale=1.0)
            nc.vector.reciprocal(rs[0:rows], rs[0:rows])
            if masked_rm is not None:
                nc.vector.tensor_mul(rs[0:rows], rs[0:rows], masked_rm)
            z = zpool.tile([128, D], bf16, tag="z")
            if rows < 128:
                nc.gpsimd.memset(z[64:128, :], 0.0)   # pad rows; LN overwrites 64:rows
            nc.vector.tensor_scalar(out=z[0:rows, :], in0=x[0:rows, :],
                                    scalar1=mv[0:rows, 0:1], scalar2=rs[0:rows],
                                    op0=OP.subtract, op1=OP.mult)
            return z

        z_keep = None
        for p in range(NPAIR):
            x = zpool.tile([128, D], bf16, tag="x")
            nc.sync.dma_start(out=x[0:72, :], in_=d_img[72 * p:72 * p + 72, :])
            rm = stp.tile([128, 1], mybir.dt.float32, tag="rm")
            nc.gpsimd.dma_start(out=rm[0:72, :], in_=d_rm[72 * p:72 * p + 72, :])
            z = ln_tile(x, 72, rm[0:72])
            z_keep = z
            if stage >= 2:
                for kin in range(4):
                    nc.sync.dma_start_transpose(
                        out=zT[:, kin, 128 * p:128 * (p + 1)],
                        in_=z[:, 128 * kin:128 * (kin + 1)])
        for t in range(3):
            rows = 128 if t < 2 else 64
            x = zpool.tile([128, D], bf16, tag="x")
            nc.sync.dma_start(out=x[0:rows, :], in_=d_cap[128 * t:128 * t + rows, :])
            z = ln_tile(x, rows, None)
            if stage >= 2:
                for kin in range(4):
                    nc.sync.dma_start_transpose(
                        out=zT[:, kin, ZT_CAP0 + 128 * t:ZT_CAP0 + 128 * (t + 1)],
                        in_=z[:, 128 * kin:128 * (kin + 1)])
        def dbg_dump(src_ap):
            dt_ = fin.tile([128, 512], mybir.dt.float32, tag="dbg")
            nc.vector.memset(dt_, 0.0)
            nc.vector.tensor_copy(out=dt_[:, 0:src_ap.free_size()], in_=src_ap)
            nc.sync.dma_start(out=d_dbg, in_=dt_)

        if stage == 1:
            dbg_dump(z_keep)
        if stage == 2:
            dbg_dump(zT[:, 0, 0:512])

        # ---- projections: kT/vtT (img, compact 36/img), qT (caps) ----
        # zT img cols: img i valid data at 128*(i//2) + 36*(i%2) .. +36
        zT_pairs = zT[:, :, 0:ZT_CAP0].rearrange("p k (a b) -> p k a b", b=128)

        for dout in range(4 if stage >= 3 else 0):
            for wname, dst in (("wk", kT), ("wvt", vtT)):
                for (i0, ni) in ICH:
                    ps = ppool.tile([128, 432], mybir.dt.float32, tag="ps")
                    for kin in range(4):
                        rhs = zT_pairs[:, kin, i0 // 2:(i0 + ni) // 2, 0:72] \
                            .rearrange("p a (c d) -> p a c d", d=36)
                        nc.tensor.matmul(ps[:, 0:ni * 36],
                                         lhsT=w_sb[wname][:, kin, 128 * dout:128 * (dout + 1)],
                                         rhs=rhs, start=(kin == 0), stop=(kin == 3))
                    nc.vector.tensor_copy(out=dst[:, dout, i0 * 36:(i0 + ni) * 36],
                                          in_=ps[:, 0:ni * 36])
            ps = ppool.tile([128, 432], mybir.dt.float32, tag="ps")
            for kin in range(4):
                nc.tensor.matmul(ps[:, 0:M],
                                 lhsT=w_sb["wq"][:, kin, 128 * dout:128 * (dout + 1)],
                                 rhs=zT[:, kin, ZT_CAP0:ZT_CAP0 + M],
                                 start=(kin == 0), stop=(kin == 3))
            nc.scalar.copy(out=qT[:, dout, :], in_=ps[:, 0:M])

        if stage == 3:
            dbg_dump(kT[:, 0, 0:512])

        # ---- per-pair Gram G3 = vt vt^T -> H[:, i, 0:36] ----
        for p in range(NPAIR if stage >= 4 else 0):
            ga = gpool.tile([128, 36], mybir.dt.float32, tag="ga")
            gb = gpool.tile([128, 36], mybir.dt.float32, tag="gb")
            for kin in range(4):
                ev = vtT[:, kin, 72 * p:72 * p + 36]
                nc.tensor.matmul(ga[0:36, :], lhsT=ev, rhs=ev,
                                 start=(kin == 0), stop=(kin == 3),
                                 skip_group_check=True)
            for kin in range(4):
                od = vtT[:, kin, 72 * p + 36:72 * p + 72]
                nc.tensor.matmul(gb[64:100, :], lhsT=od, rhs=od,
                                 start=(kin == 0), stop=(kin == 3),
                                 skip_group_check=True)
            nc.scalar.copy(out=H[0:36, 2 * p, 0:36], in_=ga[0:36, :])
            nc.scalar.copy(out=H[64:100, 2 * p + 1, 0:36], in_=gb[64:100, :])

        if stage == 4:
            dbg_dump(H[:, 0:12, :].rearrange("p a b -> p (a b)"))

        # ---- per cap-row tile: sims/T GEMMs, exp, eT, Y, reductions, finale ----
        for mt, (m0, mc) in enumerate(MT if stage >= 5 else []):
            e_sb = big.tile([128, Bi, 64], bf16, tag="e")   # 64-padded stripes
            nc.gpsimd.memset(e_sb, 0.0)
            T_sb = big.tile([128, NI], bf16, tag="T")
            for which in ("sims", "T"):
                src = kT if which == "sims" else vtT
                for (i0, ni) in ICH:
                    ps = ppool.tile([128, 432], mybir.dt.float32, tag="ps")
                    for kin in range(4):
                        nc.tensor.matmul(ps[0:mc, 0:ni * 36],
                                         lhsT=qT[:, kin, m0:m0 + mc],
                                         rhs=src[:, kin, i0 * 36:(i0 + ni) * 36],
                                         start=(kin == 0), stop=(kin == 3))
                    if which == "sims":
                        nc.scalar.activation(
                            out=e_sb[0:mc, i0:i0 + ni, 0:36],
                            in_=ps[0:mc, 0:ni * 36].rearrange("p (a b) -> p a b", b=36),
                            func=AF.Exp)
                    else:
                        nc.vector.tensor_copy(out=T_sb[0:mc, i0 * 36:(i0 + ni) * 36],
                                              in_=ps[0:mc, 0:ni * 36])
            if stage == 5:
                if mt == 0:
                    dbg_dump(e_sb[:, 0:8, :].rearrange("p a b -> p (a b)"))
                continue
            for p in range(NPAIR):
                nc.scalar.dma_start_transpose(
                    out=eT[:, p, 128 * mt:128 * (mt + 1)],
                    in_=e_sb[:, 2 * p:2 * p + 2, :])
            if stage == 6:
                if mt == 0:
                    dbg_dump(eT[:, 0, 0:384])
                continue
            # Y matmuls: per image, K=36. Concurrent PE row-groups (parity 0 at
            # partitions 0:36, parity 1 at 64:100) must drain to DIFFERENT
            # PSUM banks, so each bank holds only same-parity images.
            y3 = fin.tile([128, Bi, 36], bf16, tag="y3")
            for par in (0, 1):
                for (c0, nb) in ((0, 12), (12, 12), (24, 8)):
                    yp = ypool.tile([128, 480], mybir.dt.float32, tag="y")
                    for j in range(nb):
                        img = 2 * (c0 + j) + par
                        nc.tensor.matmul(
                            yp[0:mc, 40 * j:40 * (j + 1)],
                            lhsT=eT[64 * par:64 * par + 36, img // 2,
                                    128 * mt:128 * mt + mc],
                            rhs=H[64 * par:64 * par + 36, img, :],
                            start=True, stop=True, skip_group_check=True)
                    ypv = yp[0:mc, 0:40 * nb].rearrange("p (a b) -> p a b", b=40)
                    nc.scalar.copy(
                        out=stat3[0:mc, mt, 2 * c0 + par:2 * (c0 + nb) - 1 + par:2, :],
                        in_=ypv[:, :, 36:39])
                    nc.scalar.copy(
                        out=y3[0:mc, 2 * c0 + par:2 * (c0 + nb) - 1 + par:2, :],
                        in_=ypv[:, :, 0:36])
            if stage in (7, 71, 72):
                if mt == 0:
                    dbg_dump(y3[0:128, 0:14, :].rearrange("p a b -> p (a b)"))
                continue
            tmp = fin.tile([128, Bi, 36], bf16, tag="tmp")
            nc.vector.tensor_mul(tmp[0:mc], e_sb[0:mc, :, 0:36], y3[0:mc])
            nc.vector.reduce_sum(c3u[0:mc, mt, :], tmp[0:mc], axis=AX.X)
            tmp2 = fin.tile([128, Bi, 36], bf16, tag="tmp2")
            nc.vector.tensor_mul(tmp2[0:mc], e_sb[0:mc, :, 0:36],
                                 T_sb[0:mc, :].rearrange("p (a b) -> p a b", b=36))
            nc.vector.reduce_sum(tqu[0:mc, mt, :], tmp2[0:mc], axis=AX.X)
            # final assembly
            mu = stat3[0:mc, mt, :, 0]
            pp = stat3[0:mc, mt, :, 1]
            num = fin.tile([128, Bi], mybir.dt.float32, tag="num")
            nc.vector.scalar_tensor_tensor(out=num[0:mc], in0=mu,
                                           scalar=qw_sb[0:mc, mt, :], in1=tqu[0:mc, mt, :],
                                           op0=OP.mult, op1=OP.add)
            t1 = fin.tile([128, Bi], mybir.dt.float32, tag="t1")
            nc.vector.tensor_mul(t1[0:mc], mu, mu)
            den2 = fin.tile([128, Bi], mybir.dt.float32, tag="den2")
            nc.vector.scalar_tensor_tensor(out=den2[0:mc], in0=t1[0:mc],
                                           scalar=om_sb[0:mc], in1=c3u[0:mc, mt, :],
                                           op0=OP.mult, op1=OP.add)
            t2 = fin.tile([128, Bi], mybir.dt.float32, tag="t2")
            nc.vector.scalar_tensor_tensor(out=t2[0:mc], in0=pp, scalar=-2.0,
                                           in1=mu, op0=OP.mult, op1=OP.mult)
            nc.vector.tensor_add(den2[0:mc], den2[0:mc], t2[0:mc])
            nc.vector.tensor_scalar_max(den2[0:mc], den2[0:mc], 0.0)
            den = fin.tile([128, Bi], mybir.dt.float32, tag="den")
            nc.scalar.activation(den[0:mc], den2[0:mc], AF.Sqrt,
                                 bias=eps12[0:mc], scale=1.0)
            nc.vector.reciprocal(den[0:mc], den[0:mc])
            sres = fin.tile([128, Bi], mybir.dt.float32, tag="sres")
            nc.vector.tensor_mul(sres[0:mc], num[0:mc], den[0:mc])
            nc.vector.tensor_scalar(out=sres[0:mc], in0=sres[0:mc],
                                    scalar1=cm_sb[0:mc, mt, 0:1],
                                    scalar2=cm_sb[0:mc, mt, 1:2],
                                    op0=OP.mult, op1=OP.add)
            nc.sync.dma_start(out=d_out[m0:m0 + mc, :], in_=sres[0:mc])

    nc.compile()
    return nc


def _host_prep(imgs, caps, img_lens, cap_lens, Wq, Wk, Wv, Wo, g1, g2, g3, g4):
    import ml_dtypes
    bf = ml_dtypes.bfloat16
    scale = np.float32(1.0 / np.sqrt(D))
    Wq_eff = (g1[:, None] * Wq.T).astype(np.float32)
    Wk_eff = ((g2[:, None] * Wk.T) * scale).astype(np.float32)
    Wv_eff = (g3[:, None] * Wv.T).astype(np.float32)
    A = (g4[:, None] * Wo.T).astype(np.float32)
    Wvt_eff = (Wv_eff @ A).astype(np.float32)
    wbar = (g4 @ Wo.T).astype(np.float32)
    Omega = np.float32(wbar @ wbar)
    wv1 = (Wv_eff @ (np.ones(D, np.float32) / D)).astype(np.float32)
    wp = (Wvt_eff @ wbar).astype(np.float32)
    wqw = (Wq_eff @ wbar).astype(np.float32)

    rmask = (np.arange(R)[None, :] < img_lens[:, None]).astype(np.float32)   # (Bi,R)
    cmask = (np.arange(W)[None, :] < cap_lens[:, None]).astype(np.float32)   # (Bc,W)

    def ln_z(x):
        mu = x.mean(-1, keepdims=True, dtype=np.float32)
        xc = x - mu
        v = (xc * xc).mean(-1, keepdims=True, dtype=np.float32)
        return xc / np.sqrt(v + LN_EPS)

    z_img = (ln_z(imgs) * rmask[..., None]).astype(np.float32)   # (Bi,R,D)
    z_cap = ln_z(caps).astype(np.float32)                        # (Bc,W,D)
    m = z_img @ wv1                                              # (Bi,R)
    p = z_img @ wp
    qw = (z_cap @ wqw).reshape(Bc * W)

    mpr = np.zeros((R, Bi, 4), np.float32)
    mpr[:, :, 0] = m.T
    mpr[:, :, 1] = p.T
    mpr[:, :, 2] = rmask.T
    imgs_bf = imgs.reshape(NI, D).astype(bf)
    caps_bf = caps.reshape(Bc * W, D).astype(bf)
    cmv = np.stack([cmask.reshape(Bc * W),
                    cmask.reshape(Bc * W) - 1.0], axis=1).astype(np.float32)
    return dict(
        imgs_bf=imgs_bf, caps_bf=caps_bf,
        wk=Wk_eff.astype(bf), wvt=Wvt_eff.astype(bf), wq=Wq_eff.astype(bf),
        mpr=mpr.astype(bf), rmask=rmask.reshape(NI, 1).astype(np.float32),
        nqw=(-qw).reshape(Bc * W, 1).astype(np.float32), cmv=cmv,
        omega=np.array([[Omega]], np.float32))


def kernel(imgs, caps, img_lens, cap_lens,
           Wq, bq, Wk, bk, Wv, bv, Wo, bo,
           g1, b1, g2, b2, g3, b3, g4, b4):
    global last_results
    imgs = np.asarray(imgs, np.float32)
    caps = np.asarray(caps, np.float32)
    img_lens = np.asarray(img_lens, np.int32)
    cap_lens = np.asarray(cap_lens, np.int32)
    args = [np.asarray(x, np.float32) for x in
            (Wq, bq, Wk, bk, Wv, bv, Wo, bo, g1, b1, g2, b2, g3, b3, g4, b4)]
    (Wq, bq, Wk, bk, Wv, bv, Wo, bo, g1, b1, g2, b2, g3, b3, g4, b4) = args

    if any(np.abs(b).max() > 0 for b in (bq, bk, bv, bo, b1, b2, b3, b4)):
        return _numpy_kernel(imgs, caps, img_lens, cap_lens, Wq, bq, Wk, bk,
                             Wv, bv, Wo, bo, g1, b1, g2, b2, g3, b3, g4, b4)

    prep = _host_prep(imgs, caps, img_lens, cap_lens,
                      Wq, Wk, Wv, Wo, g1, g2, g3, g4)

    ck = os.environ.get("BK_STAGE", "9")
    if ck not in _cache:
        _cache[ck] = _build_nc()
    nc = _cache[ck]

    shared = {k: v for k, v in prep.items()
              if k not in ("caps_bf", "nqw", "cmv")}
    in_maps = []
    for c in range(N_CORES):
        im = dict(shared)
        im["caps_bf"] = prep["caps_bf"][c * M:(c + 1) * M]
        im["nqw"] = prep["nqw"][c * M:(c + 1) * M]
        im["cmv"] = prep["cmv"][c * M:(c + 1) * M]
        in_maps.append(im)

    from concourse import bass_utils
    res = bass_utils.run_bass_kernel_spmd(nc, in_maps, core_ids=list(range(N_CORES)))
    last_results = res
    # assemble: per core (320, 64) -> (Bi, CS, W)
    parts = []
    for c in range(N_CORES):
        s = res.results[c]["out_s"]                    # (M, Bi)
        parts.append(s.reshape(CS, W, Bi).transpose(2, 0, 1))
    return np.concatenate(parts, axis=1).astype(np.float32)   # (Bi, Bc, W)


# ---- numpy fallback (correct for arbitrary biases; slow path) ----
def _ln(x, g, b):
    mu = x.mean(axis=-1, keepdims=True, dtype=np.float32)
    xc = x - mu
    var = np.mean(xc * xc, axis=-1, keepdims=True, dtype=np.float32)
    return xc / np.sqrt(var + LN_EPS) * g + b


def _numpy_kernel(imgs, caps, img_lens, cap_lens, Wq, bq, Wk, bk, Wv, bv,
                  Wo, bo, g1, b1, g2, b2, g3, b3, g4, b4):
    NEG = -1e30
    img_valid = np.arange(R)[None, :] < img_lens[:, None]
    cap_valid = np.arange(W)[None, :] < cap_lens[:, None]
    imgs_m = (imgs * img_valid[..., None]).astype(np.float32)
    caps_m = (caps * cap_valid[..., None]).astype(np.float32)
    lni = _ln(imgs_m, g2, b2).reshape(NI, D)
    k = (lni @ Wk.T + bk).reshape(Bi, R, D)
    v = ((lni @ Wv.T + bv) * img_valid.reshape(NI, 1)).reshape(Bi, R, D)
    scale = np.float32(1.0 / np.sqrt(D))
    outs = []
    for j in range(Bc):
        q = (_ln(caps_m[j], g1, b1) @ Wq.T + bq).astype(np.float32)   # (W, D)
        sims = (q @ k.reshape(NI, D).T).reshape(W, Bi, R) * scale
        pm = cap_valid[j][:, None, None] & img_valid[None, :, :]
        sims = np.where(pm, sims, np.float32(NEG))
        sims -= sims.max(axis=-1, keepdims=True)
        np.exp(sims, out=sims)
        sims /= sims.sum(axis=-1, keepdims=True)
        attn = np.where(pm, sims, 0.0).transpose(1, 0, 2)             # (Bi,W,R)
        ctx = np.matmul(attn, v)                                      # (Bi,W,D)
        out = _ln(ctx, g4, b4) @ Wo.T + bo
        num = np.einsum('bwd,wd->bw', out, q)
        den = np.sqrt((out * out).sum(-1)) + 1e-8
        s = num / den
        s = np.where(cap_valid[j][None, :], s, np.float32(-1.0))
        outs.append(s.astype(np.float32))
    return np.stack(outs, axis=1)                                      # (Bi,Bc,W)
